# revision 1
# baseline (speedup 1.0000x reference)
"""Trainium2 Bass kernel for MFVIConstituency mean-field iterations.

Per batch b (one NeuronCore each, 8 total):
    q = s_con;  repeat 3x:  q[i,j] = s_con[i,j] + sum_k sig(q)[j,k] * sb[i,j,k]
    out = sigmoid(q)
where sb = s_bin * mask2o, mask2o[i,j,k] = mask[i,j] & (i!=k) & (j!=k).

Host (numpy) does: masking, fp16 cast, SBUF-cache layout packing, iteration-1
sigmoid, final transpose. Device does, per iteration: fp16 tensor_tensor mul
(DVE 2x mode) -> segmented reduction split between a DVE in-place pairwise
tree (fp16 adds at 2x) and ACT activation-accumulate, then sigmoid + xbar
transposes to rebuild the sig operand layout.

On-chip layout: q is assembled transposed (QT[j,i]); j lives on partitions in
two chunks: chunk1 = j 0:128, chunk2 "packed" = j 128:192 duplicated across
both partition halves with the i-range split (p<64: i 0:96, p>=64: i 96:192)
so every DVE instruction uses all 128 partitions.
"""

import numpy as np

S = 192
B = 8
P = 128
G = 48          # i-values per slab -> slab free size G*S = 9216
NSLAB1 = 4      # chunk1: 4 slabs of 48 i-values (j in 0:128)
NSLAB2 = 2      # chunk2 packed: 96 i-per-half * 2 halves / 48
DVE_SEGS = 34   # per slab: segments reduced by the DVE tree; rest go to ACT
SLAB_ORDER = [4, 5, 0, 1, 2, 3]   # chunk2 first so its boundary work overlaps

_CACHE = {}


def _build_program():
    import concourse.tile as tile
    from concourse import mybir, bacc
    from contextlib import ExitStack

    f32, f16 = mybir.dt.float32, mybir.dt.float16
    SLAB = G * S
    Sig = None

    nc = bacc.Bacc("TRN2", target_bir_lowering=False, debug=False, num_devices=B)
    Sig = __import__("concourse.mybir", fromlist=["x"]).ActivationFunctionType.Sigmoid
    Cpy = __import__("concourse.mybir", fromlist=["x"]).ActivationFunctionType.Copy
    c1_d = nc.dram_tensor("c1", [P, NSLAB1 * SLAB], f16, kind="ExternalInput")
    c2_d = nc.dram_tensor("c2", [P, NSLAB2 * SLAB], f16, kind="ExternalInput")
    siga_d = nc.dram_tensor("siga", [P, S], f16, kind="ExternalInput")
    sigb_d = nc.dram_tensor("sigb", [P, S], f16, kind="ExternalInput")
    sconT1_d = nc.dram_tensor("sconT1", [P, S], f32, kind="ExternalInput")
    sconT2p_d = nc.dram_tensor("sconT2p", [P, 96], f32, kind="ExternalInput")
    qt_d = nc.dram_tensor("qt_out", [S, S], f32, kind="ExternalOutput")

    with tile.TileContext(nc) as tc, ExitStack() as ctx:
        cache_p = ctx.enter_context(tc.tile_pool(name="cache", bufs=1))
        small_p = ctx.enter_context(tc.tile_pool(name="small", bufs=1))
        sig_p = ctx.enter_context(tc.tile_pool(name="sig", bufs=2))
        qt_p = ctx.enter_context(tc.tile_pool(name="qt", bufs=2))
        p_p = ctx.enter_context(tc.tile_pool(name="prod", bufs=4))
        junk_p = ctx.enter_context(tc.tile_pool(name="junk", bufs=4))
        sq_p = ctx.enter_context(tc.tile_pool(name="sq", bufs=2))
        out_p = ctx.enter_context(tc.tile_pool(name="out", bufs=1))

        sconT1_t = small_p.tile([P, S], f32, tag="sc1")
        nc.scalar.dma_start(sconT1_t[:], sconT1_d.ap())
        sconT2p_t = small_p.tile([P, 96], f32, tag="sc2")
        nc.scalar.dma_start(sconT2p_t[:], sconT2p_d.ap())
        siga_t = sig_p.tile([P, S], f16, tag="siga")
        nc.scalar.dma_start(siga_t[:], siga_d.ap())
        sigb_t = sig_p.tile([P, S], f16, tag="sigb")
        nc.scalar.dma_start(sigb_t[:], sigb_d.ap())

        cts = {}
        for idx, s in enumerate(SLAB_ORDER):
            ct = cache_p.tile([P, SLAB], f16, tag=f"c{s}")
            if s < NSLAB1:
                src = c1_d.ap()[:, s * SLAB:(s + 1) * SLAB]
            else:
                src = c2_d.ap()[:, (s - NSLAB1) * SLAB:(s - NSLAB1 + 1) * SLAB]
            eng = nc.sync
            if idx < 2:
                # split first-wave loads so compute ramps sooner
                h = SLAB // 2
                eng.dma_start(ct[:, 0:h], src[:, 0:h])
                eng.dma_start(ct[:, h:SLAB], src[:, h:SLAB])
            else:
                eng.dma_start(ct[:], src)
            cts[s] = ct

        def do_slab(s, siga_t, sigb_t, qt1, qt2, split=None):
            is1 = s < NSLAB1
            sig_t = siga_t if is1 else sigb_t
            qt_t = qt1 if is1 else qt2
            base = (s if is1 else s - NSLAB1) * G
            pt = p_p.tile([P, SLAB], f16)
            p3 = pt[:].rearrange("p (g k) -> p g k", k=S)
            in0 = cts[s][:].rearrange("p (g k) -> p g k", k=S)
            in1 = sig_t[:].unsqueeze(1).broadcast_to([P, G, S])
            if split == "g":       # ramp: match the halved first-wave DMAs
                h = G // 2
                nc.vector.tensor_tensor(p3[:, 0:h, :], in0[:, 0:h, :],
                                        in1[:, 0:h, :], mybir.AluOpType.mult)
                nc.vector.tensor_tensor(p3[:, h:G, :], in0[:, h:G, :],
                                        in1[:, h:G, :], mybir.AluOpType.mult)
            elif split == "k":     # boundary: high k-columns are ready first
                nc.vector.tensor_tensor(p3[:, :, 128:S], in0[:, :, 128:S],
                                        in1[:, :, 128:S], mybir.AluOpType.mult)
                nc.vector.tensor_tensor(p3[:, :, 0:128], in0[:, :, 0:128],
                                        in1[:, :, 0:128], mybir.AluOpType.mult)
            else:
                nc.vector.tensor_tensor(p3, in0, in1, mybir.AluOpType.mult)
            d = DVE_SEGS
            if d > 0:
                w = S
                while w > 3:   # in-place fp16 pairwise tree: 192->96->...->3
                    h = w // 2
                    nc.vector.tensor_tensor(
                        p3[:, 0:d, 0:h], p3[:, 0:d, 0:h], p3[:, 0:d, h:w],
                        mybir.AluOpType.add)
                    w = h
                nc.vector.tensor_reduce(
                    qt_t[:, base:base + d], p3[:, 0:d, 0:3],
                    axis=mybir.AxisListType.X, op=mybir.AluOpType.add)
            for g in range(d, G):
                jt = junk_p.tile([P, S], f16)
                nc.scalar.activation(
                    jt[:], pt[:, g * S:(g + 1) * S], Cpy,
                    accum_out=qt_t[:, base + g:base + g + 1])

        for it in range(3):
            qt1 = qt_p.tile([P, S], f32, tag="qt1")
            qt2 = qt_p.tile([P, 96], f32, tag="qt2")
            last = it == 2
            if not last:
                nsa = sig_p.tile([P, S], f16, tag="siga")
                nsb = sig_p.tile([P, S], f16, tag="sigb")
                sq1 = sq_p.tile([P, 256], f16, tag="sq1")
                sq2 = sq_p.tile([P, 128], f16, tag="sq2")
                tmp1 = sq_p.tile([P, 128], f16, tag="tmp1")
                tmp2 = sq_p.tile([P, 128], f16, tag="tmp2")

            for si, s in enumerate(SLAB_ORDER[0:2]):   # chunk2 slabs first
                sp = "g" if it == 0 else ("k" if si == 0 else None)
                do_slab(s, siga_t, sigb_t, qt1, qt2, split=sp)
            nc.vector.tensor_tensor(qt2[:], qt2[:], sconT2p_t[:], mybir.AluOpType.add)
            if not last:
                # chunk2 boundary work overlaps chunk1 compute below
                nc.scalar.activation(sq2[:, 0:96], qt2[:], Sig)
                nc.scalar.activation(sq2[:, 96:128], qt2[:, 0:32], Sig)  # filler
                nc.sync.dma_start_transpose(tmp2[:], sq2[:])
                nc.scalar.dma_start(nsa[0:96, 128:192], tmp2[0:96, 0:64])
                nc.scalar.dma_start(nsa[96:128, 128:192], tmp2[0:32, 64:128])
                nc.scalar.dma_start(nsb[0:64, 128:192], tmp2[32:96, 64:128])
                nc.scalar.dma_start(nsb[64:128, 128:192], tmp2[32:96, 64:128])
            else:
                o2 = out_p.tile([P, 96], f32, tag="o2")
                nc.scalar.activation(o2[:], qt2[:], Sig)
                nc.sync.dma_start(qt_d.ap()[128:192, 0:96], o2[0:64, :])
                nc.sync.dma_start(qt_d.ap()[128:192, 96:192], o2[64:128, :])

            for s in SLAB_ORDER[2:]:            # chunk1 slabs
                do_slab(s, siga_t, sigb_t, qt1, qt2)
            nc.vector.tensor_tensor(qt1[:], qt1[:], sconT1_t[:], mybir.AluOpType.add)
            if not last:
                nc.scalar.activation(sq1[:, 0:S], qt1[:], Sig)
                nc.scalar.activation(sq1[:, S:256], qt1[:, 0:64], Sig)  # filler
                nc.sync.dma_start_transpose(nsa[0:128, 0:128], sq1[:, 0:128])
                nc.sync.dma_start_transpose(tmp1[:], sq1[:, 128:256])
                nc.scalar.dma_start(nsb[0:64, 0:128], tmp1[0:64, :])
                nc.scalar.dma_start(nsb[64:128, 0:128], tmp1[0:64, :])
                siga_t, sigb_t = nsa, nsb
            else:
                o1 = out_p.tile([P, S], f32, tag="o1")
                nc.scalar.activation(o1[:], qt1[:], Sig)
                nc.sync.dma_start(qt_d.ap()[0:128, :], o1[:])
    nc.compile()
    return nc


def _get_program():
    if "nc" not in _CACHE:
        _CACHE["nc"] = _build_program()
    return _CACHE["nc"]


def _prep_core_inputs(s_con_b, sbm16_b):
    """Per-batch input dict. sbm16_b: masked s_bin, fp16, [i, j, k]."""
    A = sbm16_b
    c1 = np.ascontiguousarray(A[:, 0:128, :].transpose(1, 0, 2)).reshape(P, S * S)
    c2a = A[0:96, 128:192, :].transpose(1, 0, 2)     # [64, 96, 192]
    c2b = A[96:192, 128:192, :].transpose(1, 0, 2)   # [64, 96, 192]
    c2 = np.ascontiguousarray(np.concatenate([c2a, c2b], 0)).reshape(P, 96 * S)
    sig1 = (1.0 / (1.0 + np.exp(-s_con_b))).astype(np.float16)   # [a, k] natural
    siga = np.ascontiguousarray(sig1[0:128])
    sigb = np.ascontiguousarray(np.concatenate([sig1[128:192]] * 2, 0))
    sconT = np.ascontiguousarray(s_con_b.T)          # [j, i]
    sconT1 = sconT[0:128].copy()
    sconT2p = np.concatenate([sconT[128:192, 0:96], sconT[128:192, 96:192]], 0).copy()
    return {"c1": c1, "c2": c2, "siga": siga, "sigb": sigb,
            "sconT1": sconT1, "sconT2p": sconT2p}


def kernel(s_con, s_bin, mask):
    from concourse.bass_utils import run_bass_kernel_spmd

    s_con = np.asarray(s_con, dtype=np.float32)
    s_bin = np.asarray(s_bin, dtype=np.float32)
    mask = np.asarray(mask)

    idx = np.arange(S)
    ne = idx[:, None] != idx[None, :]                       # [a, k]
    m2 = ne[:, None, :] & ne[None, :, :]                    # [i, j, k]
    full_mask = mask[:, :, :, None] & m2[None]              # [B, i, j, k]
    sbm16 = (s_bin * full_mask).astype(np.float16)

    nc = _get_program()
    in_maps = [_prep_core_inputs(s_con[b], sbm16[b]) for b in range(B)]
    res = run_bass_kernel_spmd(nc, in_maps, list(range(B)))
    out = np.stack([res.results[b]["qt_out"].T for b in range(B)], 0)
    return np.ascontiguousarray(out.astype(np.float32))



# revision 16
# speedup vs baseline: 7.9380x; 7.9380x over previous
"""Trainium2 Bass kernel for MFVIConstituency mean-field iterations.

Per batch b (one NeuronCore each, 8 total):
    q = s_con;  repeat 3x:  q[i,j] = s_con[i,j] + sum_k sig(q)[j,k] * sb[i,j,k]
    out = sigmoid(q)
where sb = s_bin * mask2o, mask2o[i,j,k] = mask[i,j] & (i!=k) & (j!=k).

Strategy: the contraction for output column j is a matvec
    q[:, j] = sb[:, j, :] @ sig(q)[j, :]
done on the TensorEngine as a per-column accumulation group: weights
(stationary) = per-j slices of host-packed caches w1 [k 0:128, (j, i)]
and w2 [two 64-row k-halves stacked, (jj, i)], moving operand = one
column of the transposed sigmoid tiles r1/r2 [k, j] (r2 rows
duplicated so columns j >= 96 read k 128:192 at partition base 64).
s_con lands in PSUM first via 4 identity-rhs matmuls from sconT tiles.
DMAs stream on the three DMA-capable queues (SP, ACT, Pool) in
parallel, several pieces each so iter-1 matmuls run during the stream.
Iteration boundary, pipelined by column halves: ACT sigmoid
(PSUM->SBUF), PE transposes (SBUF->PSUM), DVE copies (PSUM->SBUF)
rebuild r1/r2; the k 0:128 half rebuilds while the tail columns of the
iteration are still accumulating. Host does masking/packing and the
final sigmoid (free).
"""

import numpy as np

S = 192
B = 8
P = 128
H = 64            # half partition
JJ = 96           # w2 packed j-range (j and j+96 share a column block)

_CACHE = {}


def _build_program():
    import concourse.tile as tile
    from concourse import mybir, bacc
    from contextlib import ExitStack

    f32, f16 = mybir.dt.float32, mybir.dt.float16
    Sig = mybir.ActivationFunctionType.Sigmoid
    Cpy = mybir.ActivationFunctionType.Copy

    nc = bacc.Bacc("TRN2", target_bir_lowering=False, debug=False, num_devices=B)

    w1_d = nc.dram_tensor("w1", [P, S * S], f16, kind="ExternalInput")
    w2_d = nc.dram_tensor("w2", [P, JJ * S], f16, kind="ExternalInput")
    # packed smalls: r0a | r0b | sc1 | sc2(rows 0:64) | ident+zeros
    sm_d = nc.dram_tensor("sm", [P, 5 * S], f16, kind="ExternalInput")
    q1_d = nc.dram_tensor("q1", [P, S], f32, kind="ExternalOutput")
    q2_d = nc.dram_tensor("q2", [H, S], f32, kind="ExternalOutput")

    with tile.TileContext(nc) as tc, ExitStack() as ctx:
        w_p = ctx.enter_context(tc.tile_pool(name="w", bufs=1))
        sb_p = ctx.enter_context(tc.tile_pool(name="sb", bufs=1))
        ps_p = ctx.enter_context(tc.tile_pool(name="ps", bufs=1, space="PSUM"))
        pt_p = ctx.enter_context(tc.tile_pool(name="pt", bufs=1, space="PSUM"))

        w1 = w_p.tile([P, S * S], f16, tag="w1")
        w2 = w_p.tile([P, JJ * S], f16, tag="w2")
        sm = sb_p.tile([P, 5 * S], f16, tag="sm")
        r1 = sm[:, 0:S]
        r2 = sm[:, S:2 * S]
        sc1 = sm[:, 2 * S:3 * S]
        sc2 = sm[0:H, 3 * S:4 * S]
        ident = sm[:, 4 * S:5 * S]    # [:, 0:128] = I, [:, 128:192] = 0
        sn1 = sb_p.tile([P, S], f16, tag="sn1")
        sn2 = sb_p.tile([H, S], f16, tag="sn2")
        jk1 = sb_p.tile([P, P], f16, tag="jk1")
        jk2 = sb_p.tile([P, P], f16, tag="jk2")
        o1 = sb_p.tile([P, S], f32, tag="o1")
        o2 = sb_p.tile([H, S], f32, tag="o2")
        qA = ps_p.tile([P, S], f32, tag="qA")
        qB = ps_p.tile([H, S], f32, tag="qB")
        t1 = pt_p.tile([P, P], f16, tag="t1")
        t2 = pt_p.tile([P, P], f16, tag="t2")   # both r2 row-halves stacked
        t3 = pt_p.tile([P, H], f16, tag="t3")
        t4 = pt_p.tile([P, H], f16, tag="t4")

        # dummy sigmoid (fed by a DVE memset) absorbs the ACT activation
        # table load off the iteration-boundary critical path
        nc.vector.memset(jk1[:], 0.0)
        nc.scalar.activation(jk2[:], jk1[:], Sig)

        nc.sync.dma_start(sm[:], sm_d.ap())

        def wsl(t, d, lo, hi):
            return (t[:, lo * S:hi * S], d.ap()[:, lo * S:hi * S])

        # weight stream over the three DMA queues, a few pieces each so
        # early iter-1 matmuls overlap the stream (last piece smallest)
        for eng, pieces in (
            (nc.sync, [wsl(w1, w1_d, 0, 44), wsl(w1, w1_d, 44, 78),
                       wsl(w1, w1_d, 78, 90), wsl(w1, w1_d, 90, 96)]),
            (nc.gpsimd, [wsl(w1, w1_d, 96, 142), wsl(w1, w1_d, 142, 177),
                         wsl(w1, w1_d, 177, 186), wsl(w1, w1_d, 186, 192),
                         wsl(w2, w2_d, 90, 96)]),
            (nc.scalar, [wsl(w2, w2_d, 0, 40), wsl(w2, w2_d, 40, 70),
                         wsl(w2, w2_d, 70, 84), wsl(w2, w2_d, 84, 90)]),
        ):
            for dst, src in pieces:
                eng.dma_start(dst, src)

        w1r = w1[:].rearrange("p (j i) -> p j i", i=S)
        w2r = w2[:].rearrange("p (j i) -> p j i", i=S)

        def k2args(j):
            jj, b0 = (j, 0) if j < JJ else (j - JJ, H)
            return w2r[b0:b0 + H, jj, :], r2[b0:b0 + H, j:j + 1]

        for it in range(3):
            # s_con -> PSUM via identity-rhs matmuls: out[i, j] = sconT[j, i].
            # Exactly ONE start=True per psum tile per iteration, covering
            # ALL columns (rhs cols 128:192 are zero) -- the PE pending-zero
            # region is per-tile, so later start=False writes then
            # initialize-or-accumulate correctly.
            nc.tensor.matmul(qA[:, 0:S], sc1[:, 0:P], ident[:],
                             start=True, stop=False, skip_group_check=True)
            nc.tensor.matmul(qA[:, P:S], sc2[:, 0:P], ident[0:H, 0:H],
                             start=False, stop=False, skip_group_check=True)
            nc.tensor.matmul(qB[:, 0:S], sc1[:, P:S], ident[:],
                             start=True, stop=False, skip_group_check=True)
            nc.tensor.matmul(qB[:, P:S], sc2[:, P:S], ident[0:H, 0:H],
                             start=False, stop=False, skip_group_check=True)
            for j in range(S):
                rc1 = r1[:, j:j + 1]
                nc.tensor.matmul(qA[:, j:j + 1], w1r[:, j, 0:P], rc1,
                                 start=False, stop=False, skip_group_check=True)
                nc.tensor.matmul(qB[:, j:j + 1], w1r[:, j, P:S], rc1,
                                 start=False, stop=False, skip_group_check=True)
            for j in range(S):
                wk2, rc2 = k2args(j)
                nc.tensor.matmul(qA[:, j:j + 1], wk2[:, 0:P], rc2,
                                 start=False, stop=False, skip_group_check=True)
                nc.tensor.matmul(qB[:, j:j + 1], wk2[:, P:S], rc2,
                                 start=False, stop=False, skip_group_check=True)
            if it < 2:
                nc.scalar.activation(sn1[:], qA[:], Sig)
                nc.scalar.activation(sn2[:], qB[:], Sig)
                nc.tensor.transpose(t1[:], sn1[:, 0:P], ident[:, 0:P])
                nc.tensor.transpose(t3[:], sn2[:, 0:P], ident[0:H, 0:H])
                # r2 row-halves are duplicates: transpose twice into one
                # 128-partition psum tile, one DVE copy per column half
                nc.tensor.transpose(t2[0:H, :], sn1[:, P:S], ident[:, 0:P])
                nc.tensor.transpose(t2[H:P, :], sn1[:, P:S], ident[:, 0:P])
                nc.tensor.transpose(t4[0:H, :], sn2[:, P:S], ident[0:H, 0:H])
                nc.tensor.transpose(t4[H:P, :], sn2[:, P:S], ident[0:H, 0:H])
                nc.vector.tensor_scalar_add(r1[:, 0:P], t1[:], 0.0)
                nc.vector.tensor_scalar_add(r1[:, P:S], t3[:], 0.0)
                nc.vector.tensor_scalar_add(r2[:, 0:P], t2[:], 0.0)
                nc.vector.tensor_scalar_add(r2[:, P:S], t4[:], 0.0)
            else:
                # o1 on ACT, o2 on DVE in parallel; q1 DMA on SP, q2 on ACT
                nc.scalar.activation(o1[:], qA[:], Cpy)
                nc.vector.tensor_scalar_add(o2[:], qB[:], 0.0)
                nc.sync.dma_start(q1_d.ap(), o1[:])
                nc.scalar.dma_start(q2_d.ap(), o2[:])
    nc.compile()
    return nc


def _get_program():
    if "nc" not in _CACHE:
        _CACHE["nc"] = _build_program()
    return _CACHE["nc"]


def _prep_core_inputs(s_con_b, sbm16_b, ident):
    """Per-batch input dict. sbm16_b: masked s_bin, fp16, [i, j, k]."""
    T = sbm16_b.transpose(2, 1, 0)                   # [k, j, i]
    w1 = np.ascontiguousarray(T[0:P]).reshape(P, S * S)
    T2 = T[P:S]                                      # [64, j, i]
    w2 = np.ascontiguousarray(
        np.concatenate([T2[:, 0:JJ], T2[:, JJ:S]], 0)).reshape(P, JJ * S)
    sconT = s_con_b.T.astype(np.float16)             # [j, i]
    sig0T = (1.0 / (1.0 + np.exp(-s_con_b))).T.astype(np.float16)  # [k, j]
    sm = np.zeros((P, 5 * S), dtype=np.float16)
    sm[:, 0:S] = sig0T[0:P]
    sm[0:H, S:2 * S] = sig0T[P:S]
    sm[H:P, S:2 * S] = sig0T[P:S]
    sm[:, 2 * S:3 * S] = sconT[0:P]
    sm[0:H, 3 * S:4 * S] = sconT[P:S]
    sm[:, 4 * S:4 * S + P] = ident
    return {"w1": w1, "w2": w2, "sm": sm}


def kernel(s_con, s_bin, mask):
    from concourse.bass_utils import run_bass_kernel_spmd

    s_con = np.asarray(s_con, dtype=np.float32)
    s_bin = np.asarray(s_bin, dtype=np.float32)
    mask = np.asarray(mask)

    idx = np.arange(S)
    ne = idx[:, None] != idx[None, :]                       # [a, k]
    m2 = ne[:, None, :] & ne[None, :, :]                    # [i, j, k]
    full_mask = mask[:, :, :, None] & m2[None]              # [B, i, j, k]
    sbm16 = (s_bin * full_mask).astype(np.float16)

    ident = np.eye(P, dtype=np.float16)
    nc = _get_program()
    in_maps = [_prep_core_inputs(s_con[b], sbm16[b], ident) for b in range(B)]
    res = run_bass_kernel_spmd(nc, in_maps, list(range(B)))
    out = np.empty((B, S, S), dtype=np.float32)
    for b in range(B):
        q = np.concatenate([res.results[b]["q1"], res.results[b]["q2"]], 0)
        out[b] = 1.0 / (1.0 + np.exp(-q))
    return out


# revision 23
# speedup vs baseline: 7.9829x; 1.0057x over previous
"""Trainium2 Bass kernel for MFVIConstituency mean-field iterations.

Per batch b (one NeuronCore each, 8 total):
    q = s_con;  repeat 3x:  q[i,j] = s_con[i,j] + sum_k sig(q)[j,k] * sb[i,j,k]
    out = sigmoid(q)
where sb = s_bin * mask2o, mask2o[i,j,k] = mask[i,j] & (i!=k) & (j!=k).

Strategy: the contraction for output column j is a matvec
    q[:, j] = sb[:, j, :] @ sig(q)[j, :]
done on the TensorEngine as a per-column accumulation group: weights
(stationary) = per-j slices of host-packed caches w1 [k 0:128, (j, i)]
and w2 [two 64-row k-halves stacked, (jj, i)], moving operand = one
column of the transposed sigmoid tiles r1/r2 [k, j] (r2 rows
duplicated so columns j >= 96 read k 128:192 at partition base 64).
s_con lands in PSUM first via 4 identity-rhs matmuls from sconT tiles.
DMAs stream on the three DMA-capable queues (SP, ACT, Pool) in
parallel, several pieces each so iter-1 matmuls run during the stream.
Iteration boundary, pipelined by column halves: ACT sigmoid
(PSUM->SBUF), PE transposes (SBUF->PSUM), DVE copies (PSUM->SBUF)
rebuild r1/r2. Host does masking/packing and the final
sigmoid (free).
"""

import numpy as np

S = 192
B = 8
P = 128
H = 64            # half partition
JJ = 96           # w2 packed j-range (j and j+96 share a column block)

_CACHE = {}


def _build_program():
    import concourse.tile as tile
    from concourse import mybir, bacc
    from contextlib import ExitStack

    f32, f16 = mybir.dt.float32, mybir.dt.float16
    Sig = mybir.ActivationFunctionType.Sigmoid
    Cpy = mybir.ActivationFunctionType.Copy

    nc = bacc.Bacc("TRN2", target_bir_lowering=False, debug=False, num_devices=B)

    w1_d = nc.dram_tensor("w1", [P, S * S], f16, kind="ExternalInput")
    w2_d = nc.dram_tensor("w2", [P, JJ * S], f16, kind="ExternalInput")
    # packed smalls: r0a | r0b | sc1 | sc2(rows 0:64) | ident+zeros
    sm_d = nc.dram_tensor("sm", [P, 5 * S], f16, kind="ExternalInput")
    q1_d = nc.dram_tensor("q1", [P, S], f32, kind="ExternalOutput")
    q2_d = nc.dram_tensor("q2", [H, S], f32, kind="ExternalOutput")

    with tile.TileContext(nc) as tc, ExitStack() as ctx:
        w_p = ctx.enter_context(tc.tile_pool(name="w", bufs=1))
        sb_p = ctx.enter_context(tc.tile_pool(name="sb", bufs=1))
        ps_p = ctx.enter_context(tc.tile_pool(name="ps", bufs=1, space="PSUM"))
        pt_p = ctx.enter_context(tc.tile_pool(name="pt", bufs=1, space="PSUM"))

        w1 = w_p.tile([P, S * S], f16, tag="w1")
        w2 = w_p.tile([P, JJ * S], f16, tag="w2")
        sm = sb_p.tile([P, 5 * S], f16, tag="sm")
        r1 = sm[:, 0:S]
        r2 = sm[:, S:2 * S]
        sc1 = sm[:, 2 * S:3 * S]
        sc2 = sm[0:H, 3 * S:4 * S]
        ident = sm[:, 4 * S:5 * S]    # [:, 0:128] = I, [:, 128:192] = 0
        sn1 = sb_p.tile([P, S], f16, tag="sn1")
        sn2 = sb_p.tile([H, S], f16, tag="sn2")
        jk1 = sb_p.tile([P, P], f16, tag="jk1")
        jk2 = sb_p.tile([P, P], f16, tag="jk2")
        o1 = sb_p.tile([P, S], f32, tag="o1")
        o2 = sb_p.tile([H, S], f32, tag="o2")
        qA = ps_p.tile([P, S], f32, tag="qA")
        qB = ps_p.tile([H, S], f32, tag="qB")
        t1 = pt_p.tile([P, P], f16, tag="t1")
        t2 = pt_p.tile([P, P], f16, tag="t2")   # both r2 row-halves stacked
        t3 = pt_p.tile([P, H], f16, tag="t3")
        t4 = pt_p.tile([P, H], f16, tag="t4")

        # dummy sigmoid (fed by a DVE memset) absorbs the ACT activation
        # table load off the iteration-boundary critical path
        nc.vector.memset(jk1[:], 0.0)
        nc.scalar.activation(jk2[:], jk1[:], Sig)

        nc.sync.dma_start(sm[:], sm_d.ap())

        def wsl(t, d, lo, hi):
            return (t[:, lo * S:hi * S], d.ap()[:, lo * S:hi * S])

        # weight stream over the three DMA queues, a few pieces each so
        # early iter-1 matmuls overlap the stream (last piece smallest)
        for eng, pieces in (
            (nc.sync, [wsl(w1, w1_d, 0, 44), wsl(w1, w1_d, 44, 78),
                       wsl(w1, w1_d, 78, 90), wsl(w1, w1_d, 90, 96)]),
            (nc.gpsimd, [wsl(w1, w1_d, 96, 142), wsl(w1, w1_d, 142, 177),
                         wsl(w1, w1_d, 177, 186), wsl(w1, w1_d, 186, 192),
                         wsl(w2, w2_d, 90, 96)]),
            (nc.scalar, [wsl(w2, w2_d, 0, 40), wsl(w2, w2_d, 40, 70),
                         wsl(w2, w2_d, 70, 84), wsl(w2, w2_d, 84, 90)]),
        ):
            for dst, src in pieces:
                eng.dma_start(dst, src)

        w1r = w1[:].rearrange("p (j i) -> p j i", i=S)
        w2r = w2[:].rearrange("p (j i) -> p j i", i=S)

        def k2args(j):
            jj, b0 = (j, 0) if j < JJ else (j - JJ, H)
            return w2r[b0:b0 + H, jj, :], r2[b0:b0 + H, j:j + 1]

        for it in range(3):
            # s_con -> PSUM via identity-rhs matmuls: out[i, j] = sconT[j, i].
            # Exactly ONE start=True per psum tile per iteration, covering
            # ALL columns (rhs cols 128:192 are zero) -- the PE pending-zero
            # region is per-tile, so later start=False writes then
            # initialize-or-accumulate correctly.
            nc.tensor.matmul(qA[:, 0:S], sc1[:, 0:P], ident[:],
                             start=True, stop=False, skip_group_check=True)
            nc.tensor.matmul(qA[:, P:S], sc2[:, 0:P], ident[0:H, 0:H],
                             start=False, stop=False, skip_group_check=True)
            nc.tensor.matmul(qB[:, 0:S], sc1[:, P:S], ident[:],
                             start=True, stop=False, skip_group_check=True)
            nc.tensor.matmul(qB[:, P:S], sc2[:, P:S], ident[0:H, 0:H],
                             start=False, stop=False, skip_group_check=True)
            for j in range(S):
                rc1 = r1[:, j:j + 1]
                nc.tensor.matmul(qA[:, j:j + 1], w1r[:, j, 0:P], rc1,
                                 start=False, stop=False, skip_group_check=True)
                nc.tensor.matmul(qB[:, j:j + 1], w1r[:, j, P:S], rc1,
                                 start=False, stop=False, skip_group_check=True)
            for j in range(S):
                wk2, rc2 = k2args(j)
                nc.tensor.matmul(qA[:, j:j + 1], wk2[:, 0:P], rc2,
                                 start=False, stop=False, skip_group_check=True)
                nc.tensor.matmul(qB[:, j:j + 1], wk2[:, P:S], rc2,
                                 start=False, stop=False, skip_group_check=True)
            if it < 2:
                nc.scalar.activation(sn1[:], qA[:], Sig)
                nc.scalar.activation(sn2[:], qB[:], Sig)
                nc.tensor.transpose(t1[:], sn1[:, 0:P], ident[:, 0:P])
                nc.tensor.transpose(t3[:], sn2[:, 0:P], ident[0:H, 0:H])
                # r2 row-halves are duplicates: transpose twice into one
                # 128-partition psum tile, one copy per column half
                nc.tensor.transpose(t2[0:H, :], sn1[:, P:S], ident[:, 0:P])
                nc.tensor.transpose(t2[H:P, :], sn1[:, P:S], ident[:, 0:P])
                nc.tensor.transpose(t4[0:H, :], sn2[:, P:S], ident[0:H, 0:H])
                nc.tensor.transpose(t4[H:P, :], sn2[:, P:S], ident[0:H, 0:H])
                nc.vector.tensor_scalar_add(r1[:, 0:P], t1[:], 0.0)
                nc.vector.tensor_scalar_add(r1[:, P:S], t3[:], 0.0)
                nc.scalar.activation(r2[:, 0:P], t2[:], Cpy)
                nc.scalar.activation(r2[:, P:S], t4[:], Cpy)
            else:
                # o1 on ACT, o2 on DVE in parallel; q1 DMA on SP, q2 on ACT
                nc.scalar.activation(o1[:], qA[:], Cpy)
                nc.vector.tensor_scalar_add(o2[:], qB[:], 0.0)
                nc.sync.dma_start(q1_d.ap(), o1[:])
                nc.scalar.dma_start(q2_d.ap(), o2[:])
    nc.compile()
    return nc


def _get_program():
    if "nc" not in _CACHE:
        _CACHE["nc"] = _build_program()
    return _CACHE["nc"]


def _prep_core_inputs(s_con_b, sbm16_b, ident):
    """Per-batch input dict. sbm16_b: masked s_bin, fp16, [i, j, k]."""
    T = sbm16_b.transpose(2, 1, 0)                   # [k, j, i]
    w1 = np.ascontiguousarray(T[0:P]).reshape(P, S * S)
    T2 = T[P:S]                                      # [64, j, i]
    w2 = np.ascontiguousarray(
        np.concatenate([T2[:, 0:JJ], T2[:, JJ:S]], 0)).reshape(P, JJ * S)
    sconT = s_con_b.T.astype(np.float16)             # [j, i]
    sig0T = (1.0 / (1.0 + np.exp(-s_con_b))).T.astype(np.float16)  # [k, j]
    sm = np.zeros((P, 5 * S), dtype=np.float16)
    sm[:, 0:S] = sig0T[0:P]
    sm[0:H, S:2 * S] = sig0T[P:S]
    sm[H:P, S:2 * S] = sig0T[P:S]
    sm[:, 2 * S:3 * S] = sconT[0:P]
    sm[0:H, 3 * S:4 * S] = sconT[P:S]
    sm[:, 4 * S:4 * S + P] = ident
    return {"w1": w1, "w2": w2, "sm": sm}


def kernel(s_con, s_bin, mask):
    from concourse.bass_utils import run_bass_kernel_spmd

    s_con = np.asarray(s_con, dtype=np.float32)
    s_bin = np.asarray(s_bin, dtype=np.float32)
    mask = np.asarray(mask)

    idx = np.arange(S)
    ne = idx[:, None] != idx[None, :]                       # [a, k]
    m2 = ne[:, None, :] & ne[None, :, :]                    # [i, j, k]
    full_mask = mask[:, :, :, None] & m2[None]              # [B, i, j, k]
    sbm16 = (s_bin * full_mask).astype(np.float16)

    ident = np.eye(P, dtype=np.float16)
    nc = _get_program()
    in_maps = [_prep_core_inputs(s_con[b], sbm16[b], ident) for b in range(B)]
    res = run_bass_kernel_spmd(nc, in_maps, list(range(B)))
    out = np.empty((B, S, S), dtype=np.float32)
    for b in range(B):
        q = np.concatenate([res.results[b]["q1"], res.results[b]["q2"]], 0)
        out[b] = 1.0 / (1.0 + np.exp(-q))
    return out
